# revision 8
# baseline (speedup 1.0000x reference)
"""BoundaryConvLayer GNN message-passing kernel for 8 Trainium2 NeuronCores.

Math (reference):
    alpha = relu(x @ dir_w.T + dir_b); beta = relu(x @ neu_w.T + neu_b)
    gamma = x @ rob_w.T + rob_b;       h    = x @ fc_w.T + fc_b
    agg   = segment_sum(h[row] + h[col], row)
    out   = (beta * agg + gamma) / (alpha + beta * degree + EPS)

Restructure:
    agg = count*h + (sum_col x[col]) @ fc_w.T + count*fc_b
so the per-edge gather fetches RAW x rows (fp8, 64B descriptors -- the DMA
descriptor floor) instead of h rows, and fc_w is applied once per 128-row
block after aggregation (transpose + 64-contraction matmul). No on-device
h-table build phase: the gather table is host-prepared fp8 x, so gathers
start at t=0. Segment sums run as fp8e4 DoubleRow identity matmuls (two
gather slots summed per PE pass through the doubled contraction dim).

alpha/beta stay in full f32 (matmul inputs f32): the relu sign decision
feeds a 1/(den+1e-8) denominator and fp16/f32r pre-activations flip signs
near zero-crossings, which the global-absmax error gate punishes.

Distribution: nodes sharded 8 ways by contiguous range; edges partitioned
by row owner so segment-sum stays core-local; each core gathers x[col]
rows from its own compact fp8 table (no collectives).
"""

import functools
import sys

import numpy as np

if "/opt/trn_rl_repo" not in sys.path:
    sys.path.insert(0, "/opt/trn_rl_repo")

EPS = 1e-8
P = 128


def _cfg_full():
    return dict(
        N=100_000,
        D=64,
        NCORES=8,
        GB=8,  # blocks per gather-group / phase-1b chunk
    )


def _derive(cfg):
    N, NCORES = cfg["N"], cfg["NCORES"]
    NLOC = N // NCORES
    NBLK = -(-NLOC // P)
    NLOC_PAD = NBLK * P
    cfg.update(NLOC=NLOC, NBLK=NBLK, NLOC_PAD=NLOC_PAD)
    return cfg


def _host_prep(cfg, x, edge_index, degree):
    """Build per-core input maps + unshard metadata."""
    N, D, NCORES = cfg["N"], cfg["D"], cfg["NCORES"]
    NLOC, NBLK, NLOC_PAD = cfg["NLOC"], cfg["NBLK"], cfg["NLOC_PAD"]

    x = np.asarray(x, np.float32)
    row = np.asarray(edge_index[0], np.int64)
    col = np.asarray(edge_index[1], np.int64)
    deg_in = np.asarray(degree, np.float32).reshape(-1)

    cores = []
    dmax_all = np.zeros((NCORES, NBLK), np.int64)
    for k in range(NCORES):
        base = k * NLOC
        m = (row >= base) & (row < base + NLOC)
        r = row[m] - base
        c = col[m]
        counts = np.bincount(r, minlength=NLOC)
        perm = np.argsort(-counts, kind="stable")  # local degree desc
        rank = np.empty(NLOC, np.int64)
        rank[perm] = np.arange(NLOC)
        rr = rank[r]
        order = np.argsort(rr, kind="stable")
        rs = rr[order]
        cs = c[order]
        dsort = counts[perm]
        starts = np.zeros(NLOC, np.int64)
        np.cumsum(dsort[:-1], out=starts[1:])
        occ = np.arange(len(rs)) - starts[rs]
        dpad = np.zeros(NLOC_PAD, np.int64)
        dpad[:NLOC] = dsort
        dmax_all[k] = dpad.reshape(NBLK, P).max(axis=1)
        cores.append(dict(base=base, perm=perm, rs=rs, cs=cs, occ=occ,
                          dsort=dsort))

    colw = np.maximum(dmax_all.max(axis=0), 2).astype(np.int64)
    coff = np.zeros(NBLK, np.int64)
    np.cumsum(colw[:-1], out=coff[1:])
    cfg["colw"] = [int(v) for v in colw]
    cfg["K_total"] = int(colw.sum())

    # compact per-core fp8 x tables: only gathered columns exist; uniform
    # size across cores (SPMD); last row is the all-zero pad target.
    needed_list = [np.unique(cc["cs"]) for cc in cores]
    NT_C = max(len(nd) for nd in needed_list)
    NT_PAD = -(-(NT_C + 1) // P) * P
    ZROW = NT_PAD - 1
    cfg.update(NT_PAD=NT_PAD, ZROW=ZROW)

    import concourse.mybir as mybir
    f8 = mybir.dt.np(mybir.dt.float8e4)

    in_maps = []
    for k in range(NCORES):
        cc = cores[k]
        base, perm = cc["base"], cc["perm"]
        needed = needed_list[k]

        xtab8 = np.zeros((NT_PAD, D), f8)
        xtab8[:len(needed)] = x[needed].astype(f8)

        eidx = np.full((P, cfg["K_total"]), ZROW, np.int32)
        b = cc["rs"] // P
        pp = cc["rs"] % P
        kcol = coff[b] + cc["occ"]
        eidx[pp, kcol] = np.searchsorted(needed, cc["cs"]).astype(np.int32)

        xt_loc = np.zeros((D + 1, NLOC_PAD), np.float32)
        xt_loc[:D, :NLOC] = x[base:base + NLOC][perm].T
        xt_loc[D, :NLOC] = 1.0

        dpad = np.zeros(NLOC_PAD, np.float32)
        dpad[:NLOC] = deg_in[base:base + NLOC][perm]
        degm = np.ascontiguousarray(dpad.reshape(NBLK, P).T)  # [p, b]

        cpad = np.zeros(NLOC_PAD, np.float32)
        cpad[:NLOC] = cc["dsort"]  # actual per-row edge counts
        countm = np.ascontiguousarray(cpad.reshape(NBLK, P).T)

        in_maps.append({
            "xtab8": xtab8,
            "xt_loc": xt_loc,
            "eidx": eidx,
            "degm": degm,
            "countm": countm,
            "countT16": np.ascontiguousarray(
                countm.T.reshape(1, -1)).astype(np.float16),
        })
    return in_maps, cores


def _host_weights(cfg, fc_w, fc_b, dir_w, dir_b, neu_w, neu_b, rob_w, rob_b):
    D = cfg["D"]
    import concourse.mybir as mybir
    f8 = mybir.dt.np(mybir.dt.float8e4)

    def tcat(pairs):
        w = np.zeros((D + 1, len(pairs) * D), np.float32)
        for t, (ww, bb) in enumerate(pairs):
            w[:D, t * D:(t + 1) * D] = np.asarray(ww, np.float32).T
            w[D, t * D:(t + 1) * D] = np.asarray(bb, np.float32)
        return w

    wcat_ab = tcat([(dir_w, dir_b), (neu_w, neu_b)])        # [65, 128]
    wcat_gh = tcat([(rob_w, rob_b), (fc_w, fc_b)])          # [65, 128]
    fcw16 = np.ascontiguousarray(
        np.asarray(fc_w, np.float32).T).astype(np.float16)  # [din, dout]
    fcb16 = np.asarray(fc_b, np.float32)[None, :].astype(np.float16)
    ident82 = np.concatenate([np.eye(P, dtype=np.float32)] * 2,
                             axis=1).astype(f8)             # [128, 256]
    ident16 = np.eye(P, dtype=np.float32).astype(np.float16)
    return dict(wcat_ab=wcat_ab, wcat_gh=wcat_gh, fcw16=fcw16, fcb16=fcb16,
                ident82=ident82, ident16=ident16)


def _build_nc(cfg):
    import concourse.bass as bass
    import concourse.bacc as bacc
    import concourse.mybir as mybir
    import concourse.tile as tile

    D = cfg["D"]
    NBLK, NLOC_PAD = cfg["NBLK"], cfg["NLOC_PAD"]
    NT_PAD = cfg["NT_PAD"]
    K_total, colw, GB = cfg["K_total"], cfg["colw"], cfg["GB"]
    f32, f16, i32 = mybir.dt.float32, mybir.dt.float16, mybir.dt.int32
    f8 = mybir.dt.float8e4
    coff = np.zeros(NBLK, np.int64)
    np.cumsum(np.asarray(colw[:-1]), out=coff[1:])

    nc = bacc.Bacc()
    xtab8_d = nc.declare_dram_parameter("xtab8", [NT_PAD, D], f8,
                                        isOutput=False)
    xt_loc_d = nc.declare_dram_parameter("xt_loc", [D + 1, NLOC_PAD], f32,
                                         isOutput=False)
    eidx_d = nc.declare_dram_parameter("eidx", [P, K_total], i32,
                                       isOutput=False)
    degm_d = nc.declare_dram_parameter("degm", [P, NBLK], f32, isOutput=False)
    countm_d = nc.declare_dram_parameter("countm", [P, NBLK], f32,
                                         isOutput=False)
    countT16_d = nc.declare_dram_parameter("countT16", [1, NBLK * P], f16,
                                           isOutput=False)
    wcat_ab_d = nc.declare_dram_parameter("wcat_ab", [D + 1, 2 * D], f32,
                                          isOutput=False)
    wcat_gh_d = nc.declare_dram_parameter("wcat_gh", [D + 1, 2 * D], f32,
                                          isOutput=False)
    fcw16_d = nc.declare_dram_parameter("fcw16", [D, D], f16, isOutput=False)
    fcb16_d = nc.declare_dram_parameter("fcb16", [1, D], f16, isOutput=False)
    ident82_d = nc.declare_dram_parameter("ident82", [P, 2 * P], f8,
                                          isOutput=False)
    ident16_d = nc.declare_dram_parameter("ident16", [P, P], f16,
                                          isOutput=False)
    y_d = nc.declare_dram_parameter("y", [P, NBLK * D], f32, isOutput=True)

    groups = [list(range(c0, min(c0 + GB, NBLK))) for c0 in range(0, NBLK, GB)]

    with tile.TileContext(nc) as tc:
        with (
            tc.tile_pool(name="const", bufs=1) as cp,
            tc.tile_pool(name="xtl", bufs=3) as xtlp,
            tc.tile_pool(name="msg", bufs=3) as mp,
            tc.tile_pool(name="absb", bufs=2) as abp,
            tc.tile_pool(name="ghsb", bufs=2) as ghp,
            tc.tile_pool(name="scr", bufs=6) as scp,
            tc.tile_pool(name="sx", bufs=3) as sxp,
            tc.tile_pool(name="sxT", bufs=3) as sxtp,
            tc.tile_pool(name="osb", bufs=2) as op,
            tc.tile_pool(name="psab", bufs=1, space="PSUM") as ppab,
            tc.tile_pool(name="psgh", bufs=1, space="PSUM") as ppgh,
            tc.tile_pool(name="pssx", bufs=2, space="PSUM") as ppsx,
            tc.tile_pool(name="psT", bufs=1, space="PSUM") as ppt,
            tc.tile_pool(name="psagg", bufs=2, space="PSUM") as ppagg,
        ):
            def _bodyfn():
                # ---- constants; eidx first (gathers depend on it) ----------
                eidx_sb = cp.tile([P, K_total], i32)
                nc.sync.dma_start(out=eidx_sb[:], in_=eidx_d[:])
                ident82 = cp.tile([P, 2 * P], f8)
                nc.sync.dma_start(out=ident82[:], in_=ident82_d[:])
                ident16 = cp.tile([P, P], f16)
                nc.sync.dma_start(out=ident16[:], in_=ident16_d[:])
                wab = cp.tile([D + 1, 2 * D], f32)
                nc.sync.dma_start(out=wab[:], in_=wcat_ab_d[:])
                wgh = cp.tile([D + 1, 2 * D], f32)
                nc.sync.dma_start(out=wgh[:], in_=wcat_gh_d[:])
                fcw = cp.tile([D, D], f16)
                nc.sync.dma_start(out=fcw[:], in_=fcw16_d[:])
                fcb = cp.tile([1, D], f16)
                nc.sync.dma_start(out=fcb[:], in_=fcb16_d[:])
                degm_sb = cp.tile([P, NBLK], f32)
                nc.scalar.dma_start(out=degm_sb[:], in_=degm_d[:])
                countm_sb = cp.tile([P, NBLK], f32)
                nc.scalar.dma_start(out=countm_sb[:], in_=countm_d[:])
                # countT[0, b*P + p] = edge count of (block b, partition p)
                countT = cp.tile([1, NBLK * P], f16)
                nc.scalar.dma_start(out=countT[:], in_=countT16_d[:])

                ident82v = ident82[:].rearrange("p (t m) -> p t m", t=2)

                for gi, blocks in enumerate(groups):
                    nb = len(blocks)
                    b0 = blocks[0]
                    goff = int(coff[b0])
                    Kg = int(sum(colw[b] for b in blocks))

                    # ---- gather x[col] rows (fp8, 64B descriptors) --------
                    msg = mp.tile([P, Kg * D], f8, tag="msg")
                    nc.gpsimd.indirect_dma_start(
                        out=msg[:], out_offset=None,
                        in_=xtab8_d[:],
                        in_offset=bass.IndirectOffsetOnAxis(
                            ap=eidx_sb[:, goff:goff + Kg], axis=0),
                    )

                    # ---- phase 1b for this chunk: alpha/beta/gamma/h ------
                    xt = xtlp.tile([D + 1, GB * P], f32, tag="xtl")
                    nc.sync.dma_start(
                        out=xt[:, :nb * P],
                        in_=xt_loc_d[:, P * b0:P * (b0 + nb)])
                    ps_ab = ppab.tile([P, GB * 2 * D], f32, tag="psab")
                    ps_gh = ppgh.tile([P, GB * 2 * D], f32, tag="psgh")
                    for j in range(nb):
                        nc.tensor.matmul(
                            out=ps_ab[:, j * 2 * D:(j + 1) * 2 * D],
                            lhsT=xt[:, P * j:P * (j + 1)], rhs=wab[:],
                            start=True, stop=True, skip_group_check=True)
                        nc.tensor.matmul(
                            out=ps_gh[:, j * 2 * D:(j + 1) * 2 * D],
                            lhsT=xt[:, P * j:P * (j + 1)], rhs=wgh[:],
                            start=True, stop=True, skip_group_check=True)
                    ab = abp.tile([P, GB * 2 * D], f32, tag="ab")
                    nc.scalar.activation(
                        out=ab[:, :nb * 2 * D], in_=ps_ab[:, :nb * 2 * D],
                        func=mybir.ActivationFunctionType.Relu)
                    gh = ghp.tile([P, GB * 2 * D], f32, tag="gh")
                    nc.vector.tensor_copy(out=gh[:, :nb * 2 * D],
                                          in_=ps_gh[:, :nb * 2 * D])

                    ab3 = ab[:].rearrange("p (t c) -> p t c", c=2 * D)
                    gh3 = gh[:].rearrange("p (t c) -> p t c", c=2 * D)
                    asl = ab3[:, :nb, 0:D]
                    bsl = ab3[:, :nb, D:2 * D]
                    gsl = gh3[:, :nb, 0:D]
                    hsl = gh3[:, :nb, D:2 * D]
                    degb = degm_sb[:, b0:b0 + nb].rearrange(
                        "p (t u) -> p t u", u=1).to_broadcast([P, nb, D])
                    cntb = countm_sb[:, b0:b0 + nb].rearrange(
                        "p (t u) -> p t u", u=1).to_broadcast([P, nb, D])

                    den = scp.tile([P, GB * D], f32, tag="den")
                    den3 = den[:].rearrange("p (t c) -> p t c", c=D)[:, :nb]
                    rt = scp.tile([P, GB * D], f32, tag="rt")
                    rt3 = rt[:].rearrange("p (t c) -> p t c", c=D)[:, :nb]
                    tt = scp.tile([P, GB * D], f32, tag="tt")
                    tt3 = tt[:].rearrange("p (t c) -> p t c", c=D)[:, :nb]
                    # den = relu_a + relu_b*deg + EPS ; r = 1/den
                    nc.vector.tensor_tensor(out=den3, in0=bsl, in1=degb,
                                            op=mybir.AluOpType.mult)
                    nc.vector.tensor_tensor(out=den3, in0=den3, in1=asl,
                                            op=mybir.AluOpType.add)
                    nc.vector.tensor_scalar(out=den3, in0=den3, scalar1=EPS,
                                            scalar2=None,
                                            op0=mybir.AluOpType.add)
                    nc.vector.reciprocal_approx_fast(out=rt3, in_=den3)
                    # T = count * h  (the deg*h fold of the agg)
                    nc.vector.tensor_tensor(out=tt3, in0=hsl, in1=cntb,
                                            op=mybir.AluOpType.mult)

                    # ---- segment-sum via fp8 DoubleRow identity matmuls ---
                    SXB = 4  # blocks per sx psum tile
                    aggps = ppagg.tile([P, GB * D], f32, tag="agg")
                    for s0 in range(0, nb, SXB):
                        sl = min(SXB, nb - s0)
                        ps_sx = ppsx.tile([P, SXB * D], f32, tag="pssx")
                        for bi in range(s0, s0 + sl):
                            b = blocks[bi]
                            w = int(colw[b])
                            kk = int(coff[b]) - goff
                            npair = w // 2
                            rem = w % 2
                            o = (bi - s0) * D
                            for j in range(npair):
                                rhs = msg[:, (kk + 2 * j) * D:
                                          (kk + 2 * j + 2) * D].rearrange(
                                    "p (t c) -> p t c", t=2)
                                nc.tensor.matmul(
                                    out=ps_sx[:, o:o + D],
                                    lhsT=ident82v, rhs=rhs,
                                    start=(j == 0),
                                    stop=(j == npair - 1 and rem == 0),
                                    perf_mode=mybir.MatmulPerfMode.DoubleRow,
                                    skip_group_check=True)
                            if rem:
                                nc.tensor.matmul(
                                    out=ps_sx[:, o:o + D],
                                    lhsT=ident82[:, 0:P],
                                    rhs=msg[:, (kk + w - 1) * D:
                                            (kk + w) * D],
                                    start=(npair == 0), stop=True,
                                    skip_group_check=True)
                        # drain sx -> fp16, transpose, apply fc_w
                        sx16 = sxp.tile([P, SXB * D], f16, tag="sx16")
                        if (s0 // SXB) % 2 == 0:
                            nc.scalar.copy(out=sx16[:, :sl * D],
                                           in_=ps_sx[:, :sl * D])
                        else:
                            nc.vector.tensor_copy(out=sx16[:, :sl * D],
                                                  in_=ps_sx[:, :sl * D])
                        ps_t = ppt.tile([D, SXB * P], f32, tag="psT")
                        for bi in range(s0, s0 + sl):
                            o = (bi - s0)
                            nc.tensor.matmul(
                                out=ps_t[:, o * P:(o + 1) * P],
                                lhsT=sx16[:, o * D:(o + 1) * D],
                                rhs=ident16[:], start=True, stop=True,
                                skip_group_check=True)
                        sxT = sxtp.tile([D, SXB * P], f16, tag="sxT")
                        if (s0 // SXB) % 2 == 0:
                            nc.vector.tensor_copy(out=sxT[:, :sl * P],
                                                  in_=ps_t[:, :sl * P])
                        else:
                            nc.scalar.copy(out=sxT[:, :sl * P],
                                           in_=ps_t[:, :sl * P])
                        for bi in range(s0, s0 + sl):
                            b = blocks[bi]
                            o = bi - s0
                            nc.tensor.matmul(
                                out=aggps[:, bi * D:(bi + 1) * D],
                                lhsT=sxT[:, o * P:(o + 1) * P],
                                rhs=fcw[:], start=True, stop=False,
                                skip_group_check=True)
                            nc.tensor.matmul(
                                out=aggps[:, bi * D:(bi + 1) * D],
                                lhsT=countT[0:1, b * P:(b + 1) * P],
                                rhs=fcb[:], start=False, stop=True,
                                skip_group_check=True)

                    # ---- tail: out = (beta*(T+agg) + gamma) * r -----------
                    osb = op.tile([P, GB * D], f32, tag="osb")
                    os3 = osb[:].rearrange("p (t c) -> p t c", c=D)[:, :nb]
                    ag3 = aggps[:].rearrange("p (t c) -> p t c", c=D)[:, :nb]
                    nc.vector.tensor_tensor(out=os3, in0=ag3, in1=tt3,
                                            op=mybir.AluOpType.add)
                    nc.vector.tensor_tensor(out=os3, in0=os3, in1=bsl,
                                            op=mybir.AluOpType.mult)
                    nc.vector.tensor_tensor(out=os3, in0=os3, in1=gsl,
                                            op=mybir.AluOpType.add)
                    nc.vector.tensor_tensor(out=os3, in0=os3, in1=rt3,
                                            op=mybir.AluOpType.mult)
                    nc.scalar.dma_start(
                        out=y_d[:, b0 * D:(b0 + nb) * D],
                        in_=osb[:, :nb * D])

            LOOPR = cfg.get("LOOPR", 0)
            if LOOPR:
                with tc.For_i(0, LOOPR, 1) as _i:
                    _bodyfn()
            else:
                _bodyfn()
    nc.finalize()
    return nc


_BUILD_CACHE = {}
LAST_PROFILE = {}


def _get_runner(cfg):
    """Compile the bass program once; return an executor over 8 cores."""
    key = (cfg["N"], cfg["NCORES"], tuple(cfg["colw"]), cfg["GB"],
           cfg["NT_PAD"], cfg.get("LOOPR", 0))
    if key in _BUILD_CACHE:
        return _BUILD_CACHE[key]

    import jax
    import concourse.mybir as mybir
    from jax.experimental.shard_map import shard_map
    from jax.sharding import Mesh, PartitionSpec
    from concourse.bass2jax import (
        _bass_exec_p, install_neuronx_cc_hook, partition_id_tensor)

    nc = _build_nc(cfg)
    install_neuronx_cc_hook()
    n_cores = cfg["NCORES"]
    partition_name = (nc.partition_id_tensor.name
                      if nc.partition_id_tensor else None)
    in_names, out_names, out_avals, zero_outs = [], [], [], []
    for alloc in nc.m.functions[0].allocations:
        if not isinstance(alloc, mybir.MemoryLocationSet):
            continue
        name = alloc.memorylocations[0].name
        if alloc.kind == "ExternalInput":
            if name != partition_name:
                in_names.append(name)
        elif alloc.kind == "ExternalOutput":
            out_names.append(name)
            shape = tuple(alloc.tensor_shape)
            dtype = mybir.dt.np(alloc.dtype)
            out_avals.append(jax.core.ShapedArray(shape, dtype))
            zero_outs.append(np.zeros(shape, dtype))
    n_params = len(in_names)
    n_outs = len(out_avals)
    all_names = in_names + out_names
    if partition_name is not None:
        all_names.append(partition_name)

    def _body(*args):
        operands = list(args)
        if partition_name is not None:
            operands.append(partition_id_tensor())
        return tuple(_bass_exec_p.bind(
            *operands,
            out_avals=tuple(out_avals),
            in_names=tuple(all_names),
            out_names=tuple(out_names),
            lowering_input_output_aliases=(),
            sim_require_finite=True,
            sim_require_nnan=True,
            nc=nc,
        ))

    devices = jax.devices()[:n_cores]
    mesh = Mesh(np.asarray(devices), ("core",))
    in_specs = (PartitionSpec("core"),) * (n_params + n_outs)
    out_specs = (PartitionSpec("core"),) * n_outs
    donate = tuple(range(n_params, n_params + n_outs))
    sharded = jax.jit(
        shard_map(_body, mesh=mesh, in_specs=in_specs, out_specs=out_specs,
                  check_rep=False),
        donate_argnums=donate, keep_unused=True)

    import jax.numpy as jnp
    from jax.sharding import NamedSharding
    _zshard = tuple(NamedSharding(mesh, PartitionSpec("core"))
                    for _ in zero_outs)

    @functools.partial(jax.jit, out_shardings=_zshard)
    def _mkzeros():
        return tuple(jnp.zeros((n_cores * z.shape[0], *z.shape[1:]), z.dtype)
                     for z in zero_outs)

    def run(in_maps, reps=1, async_reps=0):
        import time as _time
        per_core = [[np.asarray(m[n]) for n in in_names] for m in in_maps]
        concat_in = [np.concatenate([per_core[c][i] for c in range(n_cores)],
                                    axis=0) for i in range(n_params)]
        concat_in = [jax.device_put(a) for a in concat_in]
        for a in concat_in:
            a.block_until_ready()
        times = []
        out_arrs = None
        for _ in range(max(1, reps)):
            concat_zeros = _mkzeros()
            for z in concat_zeros:
                z.block_until_ready()
            t0 = _time.perf_counter()
            out_arrs = sharded(*concat_in, *concat_zeros)
            for o in out_arrs:
                o.block_until_ready()
            times.append(_time.perf_counter() - t0)
        results = [
            {name: np.asarray(out_arrs[i]).reshape(n_cores,
                                                   *out_avals[i].shape)[c]
             for i, name in enumerate(out_names)}
            for c in range(n_cores)
        ]
        return results, times

    _BUILD_CACHE[key] = run
    return run


def _prepare(cfg, x, edge_index, degree, fc_w, fc_b, dir_w, dir_b,
             neu_w, neu_b, rob_w, rob_b):
    x = np.asarray(x)
    in_maps, cores = _host_prep(cfg, x, edge_index, degree)
    wts = _host_weights(cfg, fc_w, fc_b, dir_w, dir_b, neu_w, neu_b,
                        rob_w, rob_b)
    for im in in_maps:
        im.update(wts)
    return in_maps, cores


def _unshard(cfg, results, cores):
    N, D, NLOC, NBLK = cfg["N"], cfg["D"], cfg["NLOC"], cfg["NBLK"]
    out = np.empty((N, D), np.float32)
    for k in range(cfg["NCORES"]):
        y2 = results[k]["y"].reshape(P, NBLK, D)
        y = np.ascontiguousarray(y2.transpose(1, 0, 2)).reshape(-1, D)[:NLOC]
        cc = cores[k]
        out[cc["base"] + cc["perm"]] = y
    return out


def kernel(x, edge_index, degree, fc_w, fc_b, dir_w, dir_b,
           neu_w, neu_b, rob_w, rob_b, _cfg=None, _reps=1, _async=0):
    cfg = _derive(dict(_cfg) if _cfg is not None else _cfg_full())
    in_maps, cores = _prepare(cfg, x, edge_index, degree, fc_w, fc_b,
                              dir_w, dir_b, neu_w, neu_b, rob_w, rob_b)
    run = _get_runner(cfg)
    results, times = run(in_maps, reps=_reps, async_reps=_async)
    LAST_PROFILE.clear()
    LAST_PROFILE["wall_times_s"] = times
    sync_times = [t for t in times if not isinstance(t, tuple)]
    LAST_PROFILE["exec_time_ns"] = int(min(sync_times) * 1e9)
    return _unshard(cfg, results, cores)


# revision 9
# speedup vs baseline: 391.0460x; 391.0460x over previous
"""BoundaryConvLayer GNN message-passing kernel for 8 Trainium2 NeuronCores.

Math (reference):
    alpha = relu(x @ dir_w.T + dir_b); beta = relu(x @ neu_w.T + neu_b)
    gamma = x @ rob_w.T + rob_b;       h    = x @ fc_w.T + fc_b
    agg   = segment_sum(h[row] + h[col], row)
    out   = (beta * agg + gamma) / (alpha + beta * degree + EPS)

Restructure:
    agg = count*h + (sum_col x[col]) @ fc_w.T + count*fc_b
so the per-edge gather fetches RAW x rows (fp8, 64B descriptors -- the DMA
descriptor floor) instead of h rows, and fc_w is applied once per 128-row
block after aggregation (transpose + 64-contraction matmul). No on-device
h-table build phase: the gather table is host-prepared fp8 x, so gathers
start at t=0. Segment sums run as fp8e4 DoubleRow identity matmuls (two
gather slots summed per PE pass through the doubled contraction dim).

alpha/beta stay in full f32 (matmul inputs f32): the relu sign decision
feeds a 1/(den+1e-8) denominator and fp16/f32r pre-activations flip signs
near zero-crossings, which the global-absmax error gate punishes.

Distribution: nodes sharded 8 ways by contiguous range; edges partitioned
by row owner so segment-sum stays core-local; each core gathers x[col]
rows from its own compact fp8 table (no collectives).
"""

import functools
import sys

import numpy as np

if "/opt/trn_rl_repo" not in sys.path:
    sys.path.insert(0, "/opt/trn_rl_repo")

EPS = 1e-8
P = 128


def _cfg_full():
    return dict(
        N=100_000,
        D=64,
        NCORES=8,
        GB=8,  # blocks per gather-group / phase-1b chunk
    )


def _derive(cfg):
    N, NCORES = cfg["N"], cfg["NCORES"]
    NLOC = N // NCORES
    NBLK = -(-NLOC // P)
    NLOC_PAD = NBLK * P
    cfg.update(NLOC=NLOC, NBLK=NBLK, NLOC_PAD=NLOC_PAD)
    return cfg


def _host_prep(cfg, x, edge_index, degree):
    """Build per-core input maps + unshard metadata."""
    N, D, NCORES = cfg["N"], cfg["D"], cfg["NCORES"]
    NLOC, NBLK, NLOC_PAD = cfg["NLOC"], cfg["NBLK"], cfg["NLOC_PAD"]

    x = np.asarray(x, np.float32)
    row = np.asarray(edge_index[0], np.int64)
    col = np.asarray(edge_index[1], np.int64)
    deg_in = np.asarray(degree, np.float32).reshape(-1)

    cores = []
    dmax_all = np.zeros((NCORES, NBLK), np.int64)
    for k in range(NCORES):
        base = k * NLOC
        m = (row >= base) & (row < base + NLOC)
        r = row[m] - base
        c = col[m]
        counts = np.bincount(r, minlength=NLOC)
        perm = np.argsort(-counts, kind="stable")  # local degree desc
        rank = np.empty(NLOC, np.int64)
        rank[perm] = np.arange(NLOC)
        rr = rank[r]
        order = np.argsort(rr, kind="stable")
        rs = rr[order]
        cs = c[order]
        dsort = counts[perm]
        starts = np.zeros(NLOC, np.int64)
        np.cumsum(dsort[:-1], out=starts[1:])
        occ = np.arange(len(rs)) - starts[rs]
        dpad = np.zeros(NLOC_PAD, np.int64)
        dpad[:NLOC] = dsort
        dmax_all[k] = dpad.reshape(NBLK, P).max(axis=1)
        cores.append(dict(base=base, perm=perm, rs=rs, cs=cs, occ=occ,
                          dsort=dsort))

    colw = np.maximum(dmax_all.max(axis=0), 2).astype(np.int64)
    coff = np.zeros(NBLK, np.int64)
    np.cumsum(colw[:-1], out=coff[1:])
    cfg["colw"] = [int(v) for v in colw]
    cfg["K_total"] = int(colw.sum())

    # compact per-core fp8 x tables: only gathered columns exist; uniform
    # size across cores (SPMD); last row is the all-zero pad target.
    needed_list = [np.unique(cc["cs"]) for cc in cores]
    NT_C = max(len(nd) for nd in needed_list)
    NT_PAD = -(-(NT_C + 1) // P) * P
    ZROW = NT_PAD - 1
    cfg.update(NT_PAD=NT_PAD, ZROW=ZROW)

    import concourse.mybir as mybir
    f8 = mybir.dt.np(mybir.dt.float8e4)

    in_maps = []
    for k in range(NCORES):
        cc = cores[k]
        base, perm = cc["base"], cc["perm"]
        needed = needed_list[k]

        xtab8 = np.zeros((NT_PAD, D), f8)
        xtab8[:len(needed)] = x[needed].astype(f8)

        eidx = np.full((P, cfg["K_total"]), ZROW, np.int32)
        b = cc["rs"] // P
        pp = cc["rs"] % P
        kcol = coff[b] + cc["occ"]
        eidx[pp, kcol] = np.searchsorted(needed, cc["cs"]).astype(np.int32)

        xt_loc = np.zeros((D + 1, NLOC_PAD), np.float32)
        xt_loc[:D, :NLOC] = x[base:base + NLOC][perm].T
        xt_loc[D, :NLOC] = 1.0

        dpad = np.zeros(NLOC_PAD, np.float32)
        dpad[:NLOC] = deg_in[base:base + NLOC][perm]
        degm = np.ascontiguousarray(dpad.reshape(NBLK, P).T)  # [p, b]

        cpad = np.zeros(NLOC_PAD, np.float32)
        cpad[:NLOC] = cc["dsort"]  # actual per-row edge counts
        countm = np.ascontiguousarray(cpad.reshape(NBLK, P).T)

        in_maps.append({
            "xtab8": xtab8,
            "xt_loc": xt_loc,
            "eidx": eidx,
            "degm": degm,
            "countm": countm,
            "countT16": np.ascontiguousarray(
                countm.T.reshape(1, -1)).astype(np.float16),
        })
    return in_maps, cores


def _host_weights(cfg, fc_w, fc_b, dir_w, dir_b, neu_w, neu_b, rob_w, rob_b):
    D = cfg["D"]
    import concourse.mybir as mybir
    f8 = mybir.dt.np(mybir.dt.float8e4)

    def tcat(pairs):
        w = np.zeros((D + 1, len(pairs) * D), np.float32)
        for t, (ww, bb) in enumerate(pairs):
            w[:D, t * D:(t + 1) * D] = np.asarray(ww, np.float32).T
            w[D, t * D:(t + 1) * D] = np.asarray(bb, np.float32)
        return w

    wcat_ab = tcat([(dir_w, dir_b), (neu_w, neu_b)])        # [65, 128]
    wcat_gh = tcat([(rob_w, rob_b), (fc_w, fc_b)])          # [65, 128]
    fcw16 = np.ascontiguousarray(
        np.asarray(fc_w, np.float32).T).astype(np.float16)  # [din, dout]
    fcb16 = np.asarray(fc_b, np.float32)[None, :].astype(np.float16)
    ident82 = np.concatenate([np.eye(P, dtype=np.float32)] * 2,
                             axis=1).astype(f8)             # [128, 256]
    ident16 = np.eye(P, dtype=np.float32).astype(np.float16)
    return dict(wcat_ab=wcat_ab, wcat_gh=wcat_gh, fcw16=fcw16, fcb16=fcb16,
                ident82=ident82, ident16=ident16)


def _build_nc(cfg):
    import concourse.bass as bass
    import concourse.bacc as bacc
    import concourse.mybir as mybir
    import concourse.tile as tile

    D = cfg["D"]
    NBLK, NLOC_PAD = cfg["NBLK"], cfg["NLOC_PAD"]
    NT_PAD = cfg["NT_PAD"]
    K_total, colw, GB = cfg["K_total"], cfg["colw"], cfg["GB"]
    f32, f16, i32 = mybir.dt.float32, mybir.dt.float16, mybir.dt.int32
    f8 = mybir.dt.float8e4
    coff = np.zeros(NBLK, np.int64)
    np.cumsum(np.asarray(colw[:-1]), out=coff[1:])

    nc = bacc.Bacc()
    xtab8_d = nc.declare_dram_parameter("xtab8", [NT_PAD, D], f8,
                                        isOutput=False)
    xt_loc_d = nc.declare_dram_parameter("xt_loc", [D + 1, NLOC_PAD], f32,
                                         isOutput=False)
    eidx_d = nc.declare_dram_parameter("eidx", [P, K_total], i32,
                                       isOutput=False)
    degm_d = nc.declare_dram_parameter("degm", [P, NBLK], f32, isOutput=False)
    countm_d = nc.declare_dram_parameter("countm", [P, NBLK], f32,
                                         isOutput=False)
    countT16_d = nc.declare_dram_parameter("countT16", [1, NBLK * P], f16,
                                           isOutput=False)
    wcat_ab_d = nc.declare_dram_parameter("wcat_ab", [D + 1, 2 * D], f32,
                                          isOutput=False)
    wcat_gh_d = nc.declare_dram_parameter("wcat_gh", [D + 1, 2 * D], f32,
                                          isOutput=False)
    fcw16_d = nc.declare_dram_parameter("fcw16", [D, D], f16, isOutput=False)
    fcb16_d = nc.declare_dram_parameter("fcb16", [1, D], f16, isOutput=False)
    ident82_d = nc.declare_dram_parameter("ident82", [P, 2 * P], f8,
                                          isOutput=False)
    ident16_d = nc.declare_dram_parameter("ident16", [P, P], f16,
                                          isOutput=False)
    y_d = nc.declare_dram_parameter("y", [P, NBLK * D], f32, isOutput=True)

    groups = [list(range(c0, min(c0 + GB, NBLK))) for c0 in range(0, NBLK, GB)]

    with tile.TileContext(nc) as tc:
        with (
            tc.tile_pool(name="const", bufs=1) as cp,
            tc.tile_pool(name="xtl", bufs=3) as xtlp,
            tc.tile_pool(name="msg", bufs=3) as mp,
            tc.tile_pool(name="absb", bufs=2) as abp,
            tc.tile_pool(name="ghsb", bufs=2) as ghp,
            tc.tile_pool(name="scr", bufs=6) as scp,
            tc.tile_pool(name="sx", bufs=3) as sxp,
            tc.tile_pool(name="sxT", bufs=3) as sxtp,
            tc.tile_pool(name="osb", bufs=2) as op,
            tc.tile_pool(name="psab", bufs=1, space="PSUM") as ppab,
            tc.tile_pool(name="psgh", bufs=1, space="PSUM") as ppgh,
            tc.tile_pool(name="pssx", bufs=2, space="PSUM") as ppsx,
            tc.tile_pool(name="psT", bufs=1, space="PSUM") as ppt,
            tc.tile_pool(name="psagg", bufs=2, space="PSUM") as ppagg,
        ):
            def _bodyfn():
                # ---- constants; eidx first (gathers depend on it) ----------
                eidx_sb = cp.tile([P, K_total], i32)
                nc.sync.dma_start(out=eidx_sb[:], in_=eidx_d[:])
                ident82 = cp.tile([P, 2 * P], f8)
                nc.sync.dma_start(out=ident82[:], in_=ident82_d[:])
                ident16 = cp.tile([P, P], f16)
                nc.sync.dma_start(out=ident16[:], in_=ident16_d[:])
                wab = cp.tile([D + 1, 2 * D], f32)
                nc.sync.dma_start(out=wab[:], in_=wcat_ab_d[:])
                wgh = cp.tile([D + 1, 2 * D], f32)
                nc.sync.dma_start(out=wgh[:], in_=wcat_gh_d[:])
                fcw = cp.tile([D, D], f16)
                nc.sync.dma_start(out=fcw[:], in_=fcw16_d[:])
                fcb = cp.tile([1, D], f16)
                nc.sync.dma_start(out=fcb[:], in_=fcb16_d[:])
                degm_sb = cp.tile([P, NBLK], f32)
                nc.scalar.dma_start(out=degm_sb[:], in_=degm_d[:])
                countm_sb = cp.tile([P, NBLK], f32)
                nc.scalar.dma_start(out=countm_sb[:], in_=countm_d[:])
                # countT[0, b*P + p] = edge count of (block b, partition p)
                countT = cp.tile([1, NBLK * P], f16)
                nc.scalar.dma_start(out=countT[:], in_=countT16_d[:])

                ident82v = ident82[:].rearrange("p (t m) -> p t m", t=2)

                for gi, blocks in enumerate(groups):
                    nb = len(blocks)
                    b0 = blocks[0]
                    goff = int(coff[b0])
                    Kg = int(sum(colw[b] for b in blocks))

                    # ---- gather x[col] rows (fp8, 64B descriptors) --------
                    msg = mp.tile([P, Kg * D], f8, tag="msg")
                    nc.gpsimd.indirect_dma_start(
                        out=msg[:], out_offset=None,
                        in_=xtab8_d[:],
                        in_offset=bass.IndirectOffsetOnAxis(
                            ap=eidx_sb[:, goff:goff + Kg], axis=0),
                    )

                    # ---- phase 1b for this chunk: alpha/beta/gamma/h ------
                    xt = xtlp.tile([D + 1, GB * P], f32, tag="xtl")
                    nc.sync.dma_start(
                        out=xt[:, :nb * P],
                        in_=xt_loc_d[:, P * b0:P * (b0 + nb)])
                    HC = 4  # blocks per ab/gh PSUM tile (1 bank each)
                    ab = abp.tile([P, GB * 2 * D], f32, tag="ab")
                    gh = ghp.tile([P, GB * 2 * D], f32, tag="gh")
                    for h0 in range(0, nb, HC):
                        hl = min(HC, nb - h0)
                        ps_ab = ppab.tile([P, HC * 2 * D], f32, tag="psab")
                        ps_gh = ppgh.tile([P, HC * 2 * D], f32, tag="psgh")
                        for jj in range(hl):
                            j = h0 + jj
                            nc.tensor.matmul(
                                out=ps_ab[:, jj * 2 * D:(jj + 1) * 2 * D],
                                lhsT=xt[:, P * j:P * (j + 1)], rhs=wab[:],
                                start=True, stop=True, skip_group_check=True)
                            nc.tensor.matmul(
                                out=ps_gh[:, jj * 2 * D:(jj + 1) * 2 * D],
                                lhsT=xt[:, P * j:P * (j + 1)], rhs=wgh[:],
                                start=True, stop=True, skip_group_check=True)
                        nc.scalar.activation(
                            out=ab[:, h0 * 2 * D:(h0 + hl) * 2 * D],
                            in_=ps_ab[:, :hl * 2 * D],
                            func=mybir.ActivationFunctionType.Relu)
                        nc.vector.tensor_copy(
                            out=gh[:, h0 * 2 * D:(h0 + hl) * 2 * D],
                            in_=ps_gh[:, :hl * 2 * D])

                    ab3 = ab[:].rearrange("p (t c) -> p t c", c=2 * D)
                    gh3 = gh[:].rearrange("p (t c) -> p t c", c=2 * D)
                    asl = ab3[:, :nb, 0:D]
                    bsl = ab3[:, :nb, D:2 * D]
                    gsl = gh3[:, :nb, 0:D]
                    hsl = gh3[:, :nb, D:2 * D]
                    degb = degm_sb[:, b0:b0 + nb].rearrange(
                        "p (t u) -> p t u", u=1).to_broadcast([P, nb, D])
                    cntb = countm_sb[:, b0:b0 + nb].rearrange(
                        "p (t u) -> p t u", u=1).to_broadcast([P, nb, D])

                    den = scp.tile([P, GB * D], f32, tag="den")
                    den3 = den[:].rearrange("p (t c) -> p t c", c=D)[:, :nb]
                    rt = scp.tile([P, GB * D], f32, tag="rt")
                    rt3 = rt[:].rearrange("p (t c) -> p t c", c=D)[:, :nb]
                    tt = scp.tile([P, GB * D], f32, tag="tt")
                    tt3 = tt[:].rearrange("p (t c) -> p t c", c=D)[:, :nb]
                    # den = relu_a + relu_b*deg + EPS ; r = 1/den
                    nc.vector.tensor_tensor(out=den3, in0=bsl, in1=degb,
                                            op=mybir.AluOpType.mult)
                    nc.vector.tensor_tensor(out=den3, in0=den3, in1=asl,
                                            op=mybir.AluOpType.add)
                    nc.vector.tensor_scalar(out=den3, in0=den3, scalar1=EPS,
                                            scalar2=None,
                                            op0=mybir.AluOpType.add)
                    nc.vector.reciprocal_approx_fast(out=rt3, in_=den3)
                    # T = count * h  (the deg*h fold of the agg)
                    nc.vector.tensor_tensor(out=tt3, in0=hsl, in1=cntb,
                                            op=mybir.AluOpType.mult)

                    # ---- segment-sum via fp8 DoubleRow identity matmuls ---
                    SXB = 4  # blocks per sx psum tile
                    aggps = ppagg.tile([P, GB * D], f32, tag="agg")
                    for s0 in range(0, nb, SXB):
                        sl = min(SXB, nb - s0)
                        ps_sx = ppsx.tile([P, SXB * D], f32, tag="pssx")
                        for bi in range(s0, s0 + sl):
                            b = blocks[bi]
                            w = int(colw[b])
                            kk = int(coff[b]) - goff
                            npair = w // 2
                            rem = w % 2
                            o = (bi - s0) * D
                            for j in range(npair):
                                rhs = msg[:, (kk + 2 * j) * D:
                                          (kk + 2 * j + 2) * D].rearrange(
                                    "p (t c) -> p t c", t=2)
                                nc.tensor.matmul(
                                    out=ps_sx[:, o:o + D],
                                    lhsT=ident82v, rhs=rhs,
                                    start=(j == 0),
                                    stop=(j == npair - 1 and rem == 0),
                                    perf_mode=mybir.MatmulPerfMode.DoubleRow,
                                    skip_group_check=True)
                            if rem:
                                nc.tensor.matmul(
                                    out=ps_sx[:, o:o + D],
                                    lhsT=ident82[:, 0:P],
                                    rhs=msg[:, (kk + w - 1) * D:
                                            (kk + w) * D],
                                    start=(npair == 0), stop=True,
                                    skip_group_check=True)
                        # drain sx -> fp16, transpose, apply fc_w
                        sx16 = sxp.tile([P, SXB * D], f16, tag="sx16")
                        if (s0 // SXB) % 2 == 0:
                            nc.scalar.copy(out=sx16[:, :sl * D],
                                           in_=ps_sx[:, :sl * D])
                        else:
                            nc.vector.tensor_copy(out=sx16[:, :sl * D],
                                                  in_=ps_sx[:, :sl * D])
                        ps_t = ppt.tile([D, SXB * P], f32, tag="psT")
                        for bi in range(s0, s0 + sl):
                            o = (bi - s0)
                            nc.tensor.matmul(
                                out=ps_t[:, o * P:(o + 1) * P],
                                lhsT=sx16[:, o * D:(o + 1) * D],
                                rhs=ident16[:], start=True, stop=True,
                                skip_group_check=True)
                        sxT = sxtp.tile([D, SXB * P], f16, tag="sxT")
                        if (s0 // SXB) % 2 == 0:
                            nc.vector.tensor_copy(out=sxT[:, :sl * P],
                                                  in_=ps_t[:, :sl * P])
                        else:
                            nc.scalar.copy(out=sxT[:, :sl * P],
                                           in_=ps_t[:, :sl * P])
                        for bi in range(s0, s0 + sl):
                            b = blocks[bi]
                            o = bi - s0
                            nc.tensor.matmul(
                                out=aggps[:, bi * D:(bi + 1) * D],
                                lhsT=sxT[:, o * P:(o + 1) * P],
                                rhs=fcw[:], start=True, stop=False,
                                skip_group_check=True)
                            nc.tensor.matmul(
                                out=aggps[:, bi * D:(bi + 1) * D],
                                lhsT=countT[0:1, b * P:(b + 1) * P],
                                rhs=fcb[:], start=False, stop=True,
                                skip_group_check=True)

                    # ---- tail: out = (beta*(T+agg) + gamma) * r -----------
                    osb = op.tile([P, GB * D], f32, tag="osb")
                    os3 = osb[:].rearrange("p (t c) -> p t c", c=D)[:, :nb]
                    ag3 = aggps[:].rearrange("p (t c) -> p t c", c=D)[:, :nb]
                    nc.vector.tensor_tensor(out=os3, in0=ag3, in1=tt3,
                                            op=mybir.AluOpType.add)
                    nc.vector.tensor_tensor(out=os3, in0=os3, in1=bsl,
                                            op=mybir.AluOpType.mult)
                    nc.vector.tensor_tensor(out=os3, in0=os3, in1=gsl,
                                            op=mybir.AluOpType.add)
                    nc.vector.tensor_tensor(out=os3, in0=os3, in1=rt3,
                                            op=mybir.AluOpType.mult)
                    nc.scalar.dma_start(
                        out=y_d[:, b0 * D:(b0 + nb) * D],
                        in_=osb[:, :nb * D])

            LOOPR = cfg.get("LOOPR", 0)
            if LOOPR:
                with tc.For_i(0, LOOPR, 1) as _i:
                    _bodyfn()
            else:
                _bodyfn()
    nc.finalize()
    return nc


_BUILD_CACHE = {}
LAST_PROFILE = {}


def _get_runner(cfg):
    """Compile the bass program once; return an executor over 8 cores."""
    key = (cfg["N"], cfg["NCORES"], tuple(cfg["colw"]), cfg["GB"],
           cfg["NT_PAD"], cfg.get("LOOPR", 0))
    if key in _BUILD_CACHE:
        return _BUILD_CACHE[key]

    import jax
    import concourse.mybir as mybir
    from jax.experimental.shard_map import shard_map
    from jax.sharding import Mesh, PartitionSpec
    from concourse.bass2jax import (
        _bass_exec_p, install_neuronx_cc_hook, partition_id_tensor)

    nc = _build_nc(cfg)
    install_neuronx_cc_hook()
    n_cores = cfg["NCORES"]
    partition_name = (nc.partition_id_tensor.name
                      if nc.partition_id_tensor else None)
    in_names, out_names, out_avals, zero_outs = [], [], [], []
    for alloc in nc.m.functions[0].allocations:
        if not isinstance(alloc, mybir.MemoryLocationSet):
            continue
        name = alloc.memorylocations[0].name
        if alloc.kind == "ExternalInput":
            if name != partition_name:
                in_names.append(name)
        elif alloc.kind == "ExternalOutput":
            out_names.append(name)
            shape = tuple(alloc.tensor_shape)
            dtype = mybir.dt.np(alloc.dtype)
            out_avals.append(jax.core.ShapedArray(shape, dtype))
            zero_outs.append(np.zeros(shape, dtype))
    n_params = len(in_names)
    n_outs = len(out_avals)
    all_names = in_names + out_names
    if partition_name is not None:
        all_names.append(partition_name)

    def _body(*args):
        operands = list(args)
        if partition_name is not None:
            operands.append(partition_id_tensor())
        return tuple(_bass_exec_p.bind(
            *operands,
            out_avals=tuple(out_avals),
            in_names=tuple(all_names),
            out_names=tuple(out_names),
            lowering_input_output_aliases=(),
            sim_require_finite=True,
            sim_require_nnan=True,
            nc=nc,
        ))

    devices = jax.devices()[:n_cores]
    mesh = Mesh(np.asarray(devices), ("core",))
    in_specs = (PartitionSpec("core"),) * (n_params + n_outs)
    out_specs = (PartitionSpec("core"),) * n_outs
    donate = tuple(range(n_params, n_params + n_outs))
    sharded = jax.jit(
        shard_map(_body, mesh=mesh, in_specs=in_specs, out_specs=out_specs,
                  check_rep=False),
        donate_argnums=donate, keep_unused=True)

    import jax.numpy as jnp
    from jax.sharding import NamedSharding
    _zshard = tuple(NamedSharding(mesh, PartitionSpec("core"))
                    for _ in zero_outs)

    @functools.partial(jax.jit, out_shardings=_zshard)
    def _mkzeros():
        return tuple(jnp.zeros((n_cores * z.shape[0], *z.shape[1:]), z.dtype)
                     for z in zero_outs)

    def run(in_maps, reps=1, async_reps=0):
        import time as _time
        per_core = [[np.asarray(m[n]) for n in in_names] for m in in_maps]
        concat_in = [np.concatenate([per_core[c][i] for c in range(n_cores)],
                                    axis=0) for i in range(n_params)]
        concat_in = [jax.device_put(a) for a in concat_in]
        for a in concat_in:
            a.block_until_ready()
        times = []
        out_arrs = None
        for _ in range(max(1, reps)):
            concat_zeros = _mkzeros()
            for z in concat_zeros:
                z.block_until_ready()
            t0 = _time.perf_counter()
            out_arrs = sharded(*concat_in, *concat_zeros)
            for o in out_arrs:
                o.block_until_ready()
            times.append(_time.perf_counter() - t0)
        results = [
            {name: np.asarray(out_arrs[i]).reshape(n_cores,
                                                   *out_avals[i].shape)[c]
             for i, name in enumerate(out_names)}
            for c in range(n_cores)
        ]
        return results, times

    _BUILD_CACHE[key] = run
    return run


def _prepare(cfg, x, edge_index, degree, fc_w, fc_b, dir_w, dir_b,
             neu_w, neu_b, rob_w, rob_b):
    x = np.asarray(x)
    in_maps, cores = _host_prep(cfg, x, edge_index, degree)
    wts = _host_weights(cfg, fc_w, fc_b, dir_w, dir_b, neu_w, neu_b,
                        rob_w, rob_b)
    for im in in_maps:
        im.update(wts)
    return in_maps, cores


def _unshard(cfg, results, cores):
    N, D, NLOC, NBLK = cfg["N"], cfg["D"], cfg["NLOC"], cfg["NBLK"]
    out = np.empty((N, D), np.float32)
    for k in range(cfg["NCORES"]):
        y2 = results[k]["y"].reshape(P, NBLK, D)
        y = np.ascontiguousarray(y2.transpose(1, 0, 2)).reshape(-1, D)[:NLOC]
        cc = cores[k]
        out[cc["base"] + cc["perm"]] = y
    return out


def kernel(x, edge_index, degree, fc_w, fc_b, dir_w, dir_b,
           neu_w, neu_b, rob_w, rob_b, _cfg=None, _reps=1, _async=0):
    cfg = _derive(dict(_cfg) if _cfg is not None else _cfg_full())
    in_maps, cores = _prepare(cfg, x, edge_index, degree, fc_w, fc_b,
                              dir_w, dir_b, neu_w, neu_b, rob_w, rob_b)
    run = _get_runner(cfg)
    results, times = run(in_maps, reps=_reps, async_reps=_async)
    LAST_PROFILE.clear()
    LAST_PROFILE["wall_times_s"] = times
    sync_times = [t for t in times if not isinstance(t, tuple)]
    LAST_PROFILE["exec_time_ns"] = int(min(sync_times) * 1e9)
    return _unshard(cfg, results, cores)
